# revision 20
# baseline (speedup 1.0000x reference)
"""Trainium2 Bass kernel for a 2-layer spiking LSTM (SLSTM) with temporal
attenuation readout.

Model (per timestep t, per batch row b):
    gates1 = x_t @ W_ih1.T + b_ih1 + mem1 @ W_hh1.T + b_hh1
    i,f,g,o = split(gates1); c1 = sig(f)*c1 + sig(i)*tanh(g); h1 = sig(o)*tanh(c1)
    mem1 = h1 - thr1*(mem1_prev > thr1);  spk1 = (mem1 > thr1)
    ... same for layer 2 with input spk1 ...
    out = (sum_t w_t * mem2_t) @ W_fc.T + b_fc,  w_t = exp(-a*(T-1-t))/Z

Sharding: data-parallel over batch B=256 across 8 cores (32 rows each);
weights replicated; the T=512 recurrence runs locally per core.

Per-core layouts (B_c = 32 batch rows/core, H = 512 = 4 chunks of 128):
  Gates PSUM tile [128, 512]: partition = (X, b) (X = output h-chunk, b =
      batch), free = (gate, hh) with gate order (i, f, o, g); h = X*128+hh.
  T-layout (state): SBUF tile [128, 128]: partition = hh, free = (X, b);
      contraction chunk kc of the next matmul = T[:, kc*32:(kc+1)*32].

All recurrent state lives ONLY in T layout (bf16): after the elementwise
chain produces h (P layout, f32), one PE transpose + one DVE op yields
mem_T = T(h) - thr*spk_T_prev directly; the spike spk_T = (T(h) >
thr*(1+spk_T_prev)) comes straight off the transpose PSUM. c1/c2 stay in P
layout fp32 (elementwise only).

Matmuls are single-term bf16 (rel-err budget 2e-2 leaves plenty of room):
the (transposed) recurrent state is the 128x32 stationary operand,
col-tiled 4x across the PE array (tile_position=(0, 32*X)); each col-strip
streams its own 512-wide slice of the host-prepacked bf16 weights, so all
128 PE columns are busy and gates land in elementwise-friendly layout.
Biases ride in via an appended ones-row (layer 1: augmented x row; layer
2: K=1 ones matmul). The temporal weighting uses a Horner recurrence
S = decay*S + mem2_T with the normalization folded into W_fc (f32).
"""

import sys

if "/opt/trn_rl_repo" not in sys.path:
    sys.path.insert(0, "/opt/trn_rl_repo")

import ml_dtypes
import numpy as np

import concourse.mybir as mybir
from concourse import bacc
from concourse.bass_utils import run_bass_kernel_spmd
from concourse.tile import TileContext

F32 = mybir.dt.float32
BF16 = mybir.dt.bfloat16
ALU = mybir.AluOpType
AFT = mybir.ActivationFunctionType
NPBF = ml_dtypes.bfloat16

ALPHA = 0.05
N_CORES = 8
H = 512
NCH = H // 128  # 4 h-chunks
BC = 32  # batch per core
# our gate order (i, f, o, g) as indices into the original (i, f, g, o)
GATE_PERM = [0, 1, 3, 2]
# filler matmul groups per step covering the two elementwise-chain PE waits
JUNK_A = 2
JUNK_B = 2


def _split_bf16(a: np.ndarray):
    """a (f32) -> (hi, lo) bf16 with hi + lo ~= a to ~2^-18 rel."""
    hi = a.astype(NPBF)
    lo = (a - hi.astype(np.float32)).astype(NPBF)
    return hi, lo


def _prep_rec_weight(W: np.ndarray) -> np.ndarray:
    """W [4H, K] -> rhs pack [128, (kc, X, gate, hh)] bf16, K = nkc*128."""
    K = W.shape[1]
    nkc = K // 128
    W4 = W.reshape(4, H, K)[GATE_PERM]  # [gate, h_out, k]
    W4 = W4.reshape(4, NCH, 128, nkc, 128)  # [gate, X, hh, kc, kk]
    W4 = W4.transpose(4, 3, 1, 0, 2)  # [kk, kc, X, gate, hh]
    return np.ascontiguousarray(W4.reshape(128, nkc * 4 * 4 * 128)).astype(NPBF)


def _prep_ih1(W_ih1: np.ndarray, bias1: np.ndarray) -> np.ndarray:
    """[4H, 14] + bias [4H] -> [15, (X, gate, hh)] bf16."""
    Wa = np.concatenate([W_ih1, bias1[:, None]], axis=1)  # [4H, 15]
    W4 = Wa.reshape(4, H, 15)[GATE_PERM].reshape(4, NCH, 128, 15)
    W4 = W4.transpose(3, 1, 0, 2)  # [k, X, gate, hh]
    return np.ascontiguousarray(W4.reshape(15, 4 * 4 * 128)).astype(NPBF)


def _prep_bias_row(bias: np.ndarray) -> np.ndarray:
    """bias [4H] -> [1, (X, gate, hh)] f32."""
    b4 = bias.reshape(4, H)[GATE_PERM].reshape(4, NCH, 128)
    b4 = b4.transpose(1, 0, 2)  # [X, gate, hh]
    return np.ascontiguousarray(b4.reshape(1, 4 * 4 * 128), np.float32)


def build_program(T: int, thr1: float, thr2: float):
    """Emit the full Bass/Tile program for one core (SPMD across 8)."""
    nc = bacc.Bacc("TRN2", target_bir_lowering=False, debug=False,
                   num_devices=N_CORES)

    def param(name, shape, dt=BF16):
        return nc.declare_dram_parameter(name, shape, dt, isOutput=False)

    x2h_d = param("x2h", [15, T * BC])
    x2l_d = param("x2l", [15, T * BC])
    wih1_d = param("wih1", [15, 2048])
    whh1_d = param("whh1", [128, NCH * 2048])
    wih2_d = param("wih2", [128, NCH * 2048])
    whh2_d = param("whh2", [128, NCH * 2048])
    b2h_d = param("b2h", [1, 2048])
    b2l_d = param("b2l", [1, 2048])
    wfc_d = param("wfc", [128, NCH * 8], F32)
    bfc_d = param("bfc", [1, 8], F32)
    idb_d = param("identb", [128, 128])
    out_d = nc.declare_dram_parameter("out", [BC, 8], F32, isOutput=True)

    decay = float(np.float32(np.exp(np.float32(-ALPHA))))

    with TileContext(nc) as tc:
        with (
            tc.tile_pool(name="const", bufs=1) as cpool,
            tc.tile_pool(name="state", bufs=1) as spool,
            tc.tile_pool(name="work", bufs=1) as wpool,
            tc.tile_pool(name="ps1", bufs=2, space="PSUM") as ps1pool,
            tc.tile_pool(name="ps2", bufs=2, space="PSUM") as ps2pool,
            tc.tile_pool(name="psx", bufs=3, space="PSUM") as psxpool,
            tc.tile_pool(name="psj", bufs=1, space="PSUM") as psjpool,
        ):
            # ---- constants into SBUF ----
            def load(dram, shape, dt=BF16, name=None):
                t = cpool.tile(shape, dt, name=name or dram.name + "_sb")
                nc.sync.dma_start(out=t[:, :], in_=dram[:, :])
                return t

            x2h = load(x2h_d, [15, T * BC])
            x2l = load(x2l_d, [15, T * BC])
            wih1 = load(wih1_d, [15, 2048])
            whh1 = load(whh1_d, [128, NCH * 2048])
            wih2 = load(wih2_d, [128, NCH * 2048])
            whh2 = load(whh2_d, [128, NCH * 2048])
            b2h = load(b2h_d, [1, 2048])
            b2l = load(b2l_d, [1, 2048])
            wfc = load(wfc_d, [128, NCH * 8], F32)
            bfc = load(bfc_d, [1, 8], F32)
            identb = load(idb_d, [128, 128])
            ones = cpool.tile([1, BC], BF16)
            nc.gpsimd.memset(ones[:, :], 1.0)

            # ---- persistent state tiles (all T-layout except c1/c2) ----
            m1T = spool.tile([128, 128], BF16)
            m2T = spool.tile([128, 128], BF16)
            spk1T = spool.tile([128, 128], BF16)
            spk2T = spool.tile([128, 128], BF16)
            rst1 = spool.tile([128, 128], F32)  # thr1*spk1T (previous step)
            rst2 = spool.tile([128, 128], F32)
            acc = spool.tile([128, 128], F32)
            gc1 = spool.tile([128, 256], F32)  # [tanh(g) | c1]
            gc2 = spool.tile([128, 256], F32)
            for t_ in (m1T, m2T, spk1T, spk2T, rst1, rst2, acc, gc1, gc2):
                nc.gpsimd.memset(t_[:, :], 0.0)

            # ---- per-step work tiles ----
            sif1 = wpool.tile([128, 256], F32)
            sif2 = wpool.tile([128, 256], F32)
            so1 = wpool.tile([128, 128], F32)
            so2 = wpool.tile([128, 128], F32)
            prod1 = wpool.tile([128, 256], F32)
            prod2 = wpool.tile([128, 256], F32)
            tc1 = wpool.tile([128, 128], F32)
            tc2 = wpool.tile([128, 128], F32)
            h1 = wpool.tile([128, 128], BF16)
            h2 = wpool.tile([128, 128], BF16)

            def mm(ps, lhsT, rhs, X, start, stop):
                nc.tensor.matmul(
                    ps[32 * X:32 * (X + 1), :], lhsT, rhs,
                    start=start, stop=stop, tile_position=(0, 32 * X),
                    skip_group_check=True)

            def rec_mms(ps, sT, w, start, stop):
                """4 kc-groups x 4 col-strips: ps += sT.T @ w (bf16)."""
                for kc in range(NCH):
                    a = sT[:, kc * 32:(kc + 1) * 32]
                    last = stop and kc == NCH - 1
                    for X in range(4):
                        mm(ps, a,
                           w[:, kc * 2048 + X * 512: kc * 2048 + (X + 1) * 512],
                           X, start and kc == 0, last)

            def cell(ps, gc, sif, so, prod, tcc, h):
                """LSTM cell elementwise from gates PSUM -> h (P layout)."""
                nc.scalar.activation(gc[:, 0:128], ps[:, 384:512], AFT.Tanh)
                nc.scalar.activation(sif[:, :], ps[:, 0:256], AFT.Sigmoid)
                nc.scalar.activation(so[:, :], ps[:, 256:384], AFT.Sigmoid)
                nc.vector.tensor_mul(prod[:, :], sif[:, :], gc[:, 0:256])
                nc.vector.tensor_add(gc[:, 128:256], prod[:, 0:128],
                                     prod[:, 128:256])
                nc.scalar.activation(tcc[:, :], gc[:, 128:256], AFT.Tanh)
                nc.vector.tensor_mul(h[:, :], so[:, :], tcc[:, :])

            def membrane(h, mT, spkT, rst, thr):
                """T(h) -> new mem_T = T(h) - rst_prev, spk_T, rst."""
                xp = psxpool.tile([128, 128], BF16, tag="xp")
                nc.tensor.transpose(xp[:, :], h[:, :], identb[:, :])
                # spk_new = (rst_prev + thr) < T(h); emitted first so the
                # layer-2 input matmuls wait only on it, not on mem_T
                nc.vector.scalar_tensor_tensor(
                    spkT[:, :], rst[:, :], thr, xp[:, :], ALU.add, ALU.is_lt)
                nc.vector.tensor_tensor(mT[:, :], xp[:, :], rst[:, :],
                                        ALU.subtract)
                nc.vector.tensor_scalar(
                    rst[:, :], spkT[:, :], thr, None, ALU.mult)

            junk = psjpool.tile([128, 512], F32)

            def junk_mms(n):
                """Dependency-free filler matmul groups (K=128, 4-way
                col-tiled = full PE array): keep the HAM activity monitor
                warm through the elementwise-chain waits. Output never
                read. Low-K fillers do NOT work — HAM watches array
                activity, and a K=1 matmul reads as idle."""
                for _ in range(n):
                    for X in range(4):
                        mm(junk, identb[:, 0:32], whh1[:, X * 512:(X + 1) * 512],
                           X, True, True)

            def l1_x_mms(t):
                """x @ W_ih1 (hi+lo) for step t -> fresh ps1 tile.
                Dependency-free: fills the PE wait for this step's h1."""
                ps1 = ps1pool.tile([128, 512], F32, tag="ps1")
                for ti, xs in enumerate((x2h, x2l)):
                    a = xs[:, t * BC:(t + 1) * BC]
                    for X in range(4):
                        mm(ps1, a, wih1[:, X * 512:(X + 1) * 512],
                           X, ti == 0, False)
                return ps1

            def l2_bias_mms():
                """Bias (hi+lo) -> fresh ps2 tile. Dependency-free fill."""
                ps2 = ps2pool.tile([128, 512], F32, tag="ps2")
                for ti, b in enumerate((b2h, b2l)):
                    for X in range(4):
                        mm(ps2, ones[:, :], b[:, X * 512:(X + 1) * 512],
                           X, ti == 0, False)
                return ps2

            # Software-rotated loop. PE FIFO per steady-state iteration:
            #   x(t+1), bias(t+1)          <- dependency-free, run during
            #   T1(t)                         the wait for h1(t)
            #   junkA                      <- covers the spk1T DVE latency
            #   L2in(t)  [ps2(t) stop]
            #   L1rec(t+1) [ps1(t+1) stop] <- runs during cell2(t) chain
            #   junkB                      <- covers the wait for h2(t)
            #   T2(t)
            #   L2rec(t+1)
            ps1 = l1_x_mms(0)
            rec_mms(ps1, m1T, whh1, False, True)
            ps2 = l2_bias_mms()
            rec_mms(ps2, m2T, whh2, False, False)
            for t in range(T):
                cell(ps1, gc1, sif1, so1, prod1, tc1, h1)
                ps1_nxt = l1_x_mms(t + 1) if t + 1 < T else None
                ps2_nxt = l2_bias_mms() if t + 1 < T else None
                membrane(h1, m1T, spk1T, rst1, thr1)
                junk_mms(JUNK_A)
                rec_mms(ps2, spk1T, wih2, False, True)  # completes ps2(t)
                ps2_cur = ps2
                if ps1_nxt is not None:
                    rec_mms(ps1_nxt, m1T, whh1, False, True)
                    ps1 = ps1_nxt

                cell(ps2_cur, gc2, sif2, so2, prod2, tc2, h2)
                junk_mms(JUNK_B)
                membrane(h2, m2T, spk2T, rst2, thr2)
                # temporal attenuation (Horner): acc = decay*acc + mem2_T
                nc.vector.scalar_tensor_tensor(
                    acc[:, :], acc[:, :], decay, m2T[:, :], ALU.mult, ALU.add)
                if ps2_nxt is not None:
                    rec_mms(ps2_nxt, m2T, whh2, False, False)
                    ps2 = ps2_nxt

            # ---------------- readout: out = acc-weighted FC ------------
            psfc = psxpool.tile([32, 8], F32, tag="xp")
            for kc in range(NCH):
                nc.tensor.matmul(
                    psfc[:, :], acc[:, kc * 32:(kc + 1) * 32],
                    wfc[:, kc * 8:(kc + 1) * 8],
                    start=(kc == 0), stop=False, skip_group_check=True)
            onesf = wpool.tile([1, BC], F32)
            nc.gpsimd.memset(onesf[:, :], 1.0)
            nc.tensor.matmul(psfc[:, :], onesf[:, :], bfc[:, :],
                             start=False, stop=True, skip_group_check=True)
            outsb = wpool.tile([32, 8], F32)
            nc.vector.tensor_copy(outsb[:, :], psfc[:, :])
            nc.sync.dma_start(out=out_d[:, :], in_=outsb[:, :])

    nc.compile()
    return nc


def prep_inputs(x, W_ih1, W_hh1, b_ih1, b_hh1, W_ih2, W_hh2, b_ih2, b_hh2,
                W_fc, b_fc, T):
    """Host-side packing into per-core in_maps."""
    x = np.asarray(x, np.float32)
    # normalization constant of the attenuation weights (folded into W_fc)
    w32 = np.exp(np.float32(-ALPHA) * np.arange(T - 1, -1, -1, dtype=np.float32))
    Z = float(np.float64(w32.sum()))

    wih1 = _prep_ih1(np.asarray(W_ih1, np.float32),
                     np.asarray(b_ih1, np.float32) + np.asarray(b_hh1, np.float32))
    whh1 = _prep_rec_weight(np.asarray(W_hh1, np.float32))
    wih2 = _prep_rec_weight(np.asarray(W_ih2, np.float32))
    whh2 = _prep_rec_weight(np.asarray(W_hh2, np.float32))
    b2h, b2l = _split_bf16(_prep_bias_row(
        np.asarray(b_ih2, np.float32) + np.asarray(b_hh2, np.float32)))
    # wfc [128, (kc, c)]: wfc[kk, kc*8+c] = W_fc[c, kc*128+kk] / Z
    wfc = (np.asarray(W_fc, np.float64) / Z).astype(np.float32)  # [8, 512]
    wfc = wfc.reshape(8, NCH, 128).transpose(2, 1, 0)
    wfc = np.ascontiguousarray(wfc.reshape(128, NCH * 8), np.float32)
    bfc = np.asarray(b_fc, np.float32).reshape(1, 8)
    identb = np.eye(128, dtype=NPBF)

    common = {"wih1": wih1, "whh1": whh1, "wih2": wih2, "whh2": whh2,
              "b2h": b2h, "b2l": b2l, "wfc": wfc, "bfc": bfc,
              "identb": identb}
    in_maps = []
    for c in range(N_CORES):
        xs = x[:, c * BC:(c + 1) * BC, :]  # [T, 32, 14]
        x_aug = np.empty((15, T * BC), np.float32)
        x_aug[:14] = xs.transpose(2, 0, 1).reshape(14, T * BC)
        x_aug[14] = 1.0
        xh, xl = _split_bf16(x_aug)
        in_maps.append({"x2h": xh, "x2l": xl, **common})
    return in_maps


_CACHE = {}


def run(trace=False, **inputs):
    """Build+run; returns (out [B, 8] float32, BassKernelResults)."""
    x = np.asarray(inputs["x"], np.float32)
    T = x.shape[0]
    thr1 = float(np.asarray(inputs["thr1"]))
    thr2 = float(np.asarray(inputs["thr2"]))
    key = (T, thr1, thr2)
    if key not in _CACHE:
        _CACHE[key] = build_program(T, thr1, thr2)
    nc = _CACHE[key]
    in_maps = prep_inputs(
        x, inputs["W_ih1"], inputs["W_hh1"], inputs["b_ih1"], inputs["b_hh1"],
        inputs["W_ih2"], inputs["W_hh2"], inputs["b_ih2"], inputs["b_hh2"],
        inputs["W_fc"], inputs["b_fc"], T)
    res = run_bass_kernel_spmd(nc, in_maps, core_ids=list(range(N_CORES)),
                               trace=trace)
    out = np.concatenate([r["out"] for r in res.results], axis=0)
    return np.ascontiguousarray(out, np.float32), res


def kernel(**inputs) -> np.ndarray:
    out, _ = run(trace=False, **inputs)
    return out
